# revision 1
# baseline (speedup 1.0000x reference)
"""Trainium2 Bass kernel for BatchWiseTripletDistanceLoss.

Math: loss = mean_t relu(cos_d(s[a_t], s[p_t]) - cos_d(s[a_t], s[n_t]) + margin)
with cos_d(x, y) = 1 - <x,y>/max(|x||y|, eps).

Cosine distances depend only on (row, row) pairs of the 512x256 sample
matrix, so the kernel computes the 512x512 cosine-SIMILARITY matrix
sim = R S S^T R (R = diag(1/|s_i|)) on-device via TensorE and evaluates
    relu(sim[a,p] - sim[a,n] + margin)          ("1-" cancels in the diff)
on a dense [row, col] grid: each triplet is scattered to grid cell
(a_t, n_t) carrying p_t+1 (gpsimd local_scatter = true per-partition
scatter).  The distinct positives of each row form a small palette
(~13 entries here); palette VALUES are extracted from the sim row by a
second local_scatter, and a short loop over palette slots evaluates
masked relu terms, so no per-triplet gather is ever needed.

Sharding: 8 cores split the grid into (row mod 4) x (column half)
quadrants of [128, 256].  The host only transposes/permutes/buckets/pads
the given arrays (layout + palette metadata, no float math) and sums the
8 partial scalars at the end.
"""
import sys

sys.path.insert(0, "/opt/trn_rl_repo")

from contextlib import ExitStack

import numpy as np
import ml_dtypes

ml_bf16 = ml_dtypes.bfloat16

import concourse.bacc as bacc
import concourse.bass as bass
import concourse.tile as tile
from concourse import mybir
from concourse.bass_utils import run_bass_kernel_spmd

DT = mybir.dt
OP = mybir.AluOpType
ACTF = mybir.ActivationFunctionType

N = 512
D = 256
MARGIN = 0.15
NCORES = 8
LCOL = 256  # columns per core (half)
NROW = 128  # rows per core (stride-4 residue class)
NCHUNK = 4  # main-loop pipeline chunks


def _build_program(s_pad: int):
    """Build + compile the SPMD program (identical for all 8 cores)."""
    nc = bacc.Bacc(
        "TRN2", target_bir_lowering=False, debug=False, num_devices=NCORES
    )
    f32, i32, i16, f16 = DT.float32, DT.int32, DT.int16, DT.float16

    WB = 256 + 512 + 256 + s_pad
    d_packa = nc.dram_tensor("packa", [128, 1280], f32, kind="ExternalInput").ap()
    d_packb = nc.dram_tensor("packb", [NROW, WB], i16, kind="ExternalInput").ap()
    d_out = nc.dram_tensor("out", [NROW, NCHUNK], f32, kind="ExternalOutput").ap()

    with tile.TileContext(nc) as tc, ExitStack() as ctx:
        cpool = ctx.enter_context(tc.tile_pool(name="const", bufs=1))
        wpool = ctx.enter_context(tc.tile_pool(name="work", bufs=2))
        mpool = ctx.enter_context(tc.tile_pool(name="mainloop", bufs=4))
        ppool = ctx.enter_context(tc.tile_pool(name="psum", bufs=2, space="PSUM"))
        pfin = ctx.enter_context(tc.tile_pool(name="psumfin", bufs=1, space="PSUM"))
        pbig = ctx.enter_context(tc.tile_pool(name="psumbig", bufs=1, space="PSUM"))

        # ---- load inputs (two packed DMAs) ------------------------------
        packa = cpool.tile([128, 1280], f32)
        nc.sync.dma_start(packa[:], d_packa)
        packb = cpool.tile([NROW, 256 + 512 + 256 + s_pad], DT.int16)
        nc.scalar.dma_start(packb[:], d_packb)
        st = [packa[:, 0:512], packa[:, 512:1024]]
        sr = [packa[:, 1024:1152], packa[:, 1152:1280]]
        nbuk16 = packb[:, 0:256]
        sidx16 = packb[:, 256:768]
        pbuk16 = packb[:, 768:1024].bitcast(DT.bfloat16)
        ranks1 = packb[:, 1024 : 1024 + s_pad].bitcast(DT.bfloat16)

        ones_col = cpool.tile([128, 1], f32)
        nc.vector.memset(ones_col[:], 1.0)
        ones_row1 = cpool.tile([1, 128], f32)
        nc.vector.memset(ones_row1[:], 1.0)

        # ---- preload ACT sqrt table during the DMA phase ----------------
        dumin = cpool.tile([1, 1], f32)
        nc.vector.memset(dumin[:], 4.0)
        dumout = cpool.tile([1, 1], f32)
        nc.scalar.sqrt(dumout[:], dumin[:])

        # ---- pidxg scatter (independent of samples) ---------------------
        pidxg = wpool.tile([NROW, LCOL], DT.bfloat16, tag="pidxg")
        nc.gpsimd.local_scatter(
            pidxg[:], pbuk16, nbuk16,
            channels=128, num_elems=LCOL, num_idxs=LCOL,
        )

        # ---- squares on DVE (early, one op) -----------------------------
        sqall = wpool.tile([128, 1280], f32, tag="sqall")
        nc.vector.tensor_tensor(sqall[:], packa[:], packa[:], OP.mult)
        sq = [sqall[:, 0:512], sqall[:, 512:1024]]
        sqr = [sqall[:, 1024:1152], sqall[:, 1152:1280]]

        # ---- PE: norm reductions first, then sim, then RB ---------------
        n2p = pbig.tile([1, N], f32, tag="n2row")
        for k in range(2):
            nc.tensor.matmul(n2p[:], ones_col[:], sq[k], start=(k == 0), stop=(k == 1))
        n2rp = ppool.tile([128, 1], f32, tag="n2rp")
        for k in range(2):
            nc.tensor.matmul(n2rp[:], sqr[k], ones_col[:], start=(k == 0), stop=(k == 1))
        simp = pbig.tile([128, N], f32, tag="simp")
        for k in range(2):
            nc.tensor.matmul(simp[:], sr[k], st[k], start=(k == 0), stop=(k == 1))

        nrow = wpool.tile([1, N], f32, tag="nrow")
        nc.scalar.sqrt(nrow[:], n2p[:])
        nrr = wpool.tile([128, 1], f32, tag="nrr")
        nc.scalar.sqrt(nrr[:], n2rp[:])
        # preload relu table right after the sqrts (hidden off critical path)
        durelu = cpool.tile([1, 1], f32)
        nc.scalar.activation(durelu[:], dumout[:], ACTF.Relu)

        rrow = wpool.tile([1, N], f32, tag="rrow")
        rscr = wpool.tile([1, N], f32, tag="rscr")
        nc.vector.reciprocal_approx_accurate(rrow[:], nrow[:], rscr[:])
        rr = cpool.tile([128, 1], f32)
        rscr2 = wpool.tile([128, 1], f32, tag="rscr2")
        nc.vector.reciprocal_approx_accurate(rr[:], nrr[:], rscr2[:])

        rbp = pbig.tile([128, N], f32, tag="rb")
        nc.tensor.matmul(rbp[:], ones_row1[:], rrow[:], start=True, stop=True)
        t0 = wpool.tile([128, N], f32, tag="t0")
        nc.scalar.activation(t0[:], simp[:], ACTF.Copy, scale=rr[:])
        simrow = cpool.tile([128, N], f32)
        nc.vector.tensor_tensor(simrow[:], t0[:], rbp[:], OP.mult)
        sim16 = cpool.tile([128, N], f16)
        nc.scalar.copy(sim16[:], simrow[:])

        # ---- palette values + margin bias -------------------------------
        palv16 = wpool.tile([128, s_pad], f16, tag="palv16")
        nc.gpsimd.local_scatter(
            palv16[:], sim16[:], sidx16,
            channels=128, num_elems=s_pad, num_idxs=N,
        )
        palvf = wpool.tile([128, s_pad], f32, tag="palvf")
        nc.scalar.copy(palvf[:], palv16[:])
        mb = wpool.tile([128, s_pad], f32, tag="mb")
        nc.vector.tensor_scalar(mb[:], palvf[:], -1.0, MARGIN, OP.mult, OP.add)

        # ---- main palette loop (batched, chunked for pipelining) --------
        bounds = [(s_pad * c) // NCHUNK for c in range(NCHUNK + 1)]
        accs = wpool.tile([128, NCHUNK], f32, tag="accs")
        for c in range(NCHUNK):
            lo, hi = bounds[c], bounds[c + 1]
            w = (hi - lo) * LCOL
            msc = mpool.tile([128, w], DT.bfloat16, tag="msc", name=f"msc{c}")
            nc.vector.tensor_tensor(
                msc[:].rearrange("p (s j) -> p s j", s=hi - lo),
                pidxg[:].unsqueeze(1).to_broadcast((NROW, hi - lo, LCOL)),
                ranks1[:, lo:hi].unsqueeze(2).to_broadcast((NROW, hi - lo, LCOL)),
                OP.is_equal,
            )
            t1c = mpool.tile([128, w], f32, tag="t1c", name=f"t1c{c}")
            nc.vector.tensor_tensor(
                t1c[:].rearrange("p (s j) -> p s j", s=hi - lo),
                simrow[:, 0:LCOL].unsqueeze(1).to_broadcast((NROW, hi - lo, LCOL)),
                mb[:, lo:hi].unsqueeze(2).to_broadcast((NROW, hi - lo, LCOL)),
                OP.add,
            )
            mkc = mpool.tile([128, w], f32, tag="mkc", name=f"mkc{c}")
            eng = nc.gpsimd if c < 2 else nc.vector
            eng.tensor_tensor(mkc[:], msc[:], t1c[:], OP.mult)
            rlc = mpool.tile([128, w], f32, tag="rlc", name=f"rlc{c}")
            nc.scalar.activation(
                rlc[:], mkc[:], ACTF.Relu, accum_out=accs[:, c : c + 1]
            )
        nc.sync.dma_start(d_out, accs[:])

    nc.compile()
    return nc


_PROGRAM_CACHE = {}


def _get_program(s_pad):
    if s_pad not in _PROGRAM_CACHE:
        _PROGRAM_CACHE[s_pad] = _build_program(s_pad)
    return _PROGRAM_CACHE[s_pad]


def _shard_inputs(samples, targets, a, p, n, s_pad):
    """Per-core layout: transpose/permute samples, bucket triplets, build
    palette metadata (distinct positives per row)."""
    in_maps = []
    for core in range(NCORES):
        R, H = core >> 1, core & 1
        rows = np.arange(NROW, dtype=np.int64) * 4 + R
        perm = np.concatenate(
            [np.arange(256 * H, 256 * H + 256), np.arange(256 * (1 - H), 256 * (2 - H))]
        )
        sel = ((a & 3) == R) & ((n >> 8) == H)
        asel, psel, nsel = a[sel], p[sel], n[sel]
        q = asel >> 2
        order = np.argsort(q, kind="stable")
        qs = q[order]
        counts = np.bincount(qs, minlength=NROW)
        if counts.max() > LCOL:
            raise ValueError("bucket overflow")
        starts = np.zeros(NROW, dtype=np.int64)
        starts[1:] = np.cumsum(counts)[:-1]
        slot = np.arange(len(qs)) - starts[qs]
        nbuk = np.full((NROW, LCOL), -1, dtype=np.int16)
        nbuk[qs, slot] = (nsel[order] & 255).astype(np.int16)

        # palettes: distinct positives per row; local col of raw id v:
        # (v & 255) + 256 * (v >> 8 != H)
        sidx = np.full((NROW, N), -1, dtype=np.int16)
        palidx1 = np.full((NROW, s_pad), -1.0, dtype=np.float32)  # -1 matches nothing
        rankof = {}
        ar = a[(a & 3) == R]
        pr = p[(a & 3) == R]
        rr_ = ar >> 2
        for qq in range(NROW):
            vals = np.unique(pr[rr_ == qq])
            if len(vals) > s_pad:
                raise ValueError("palette overflow")
            if len(vals) == 0:
                continue
            lcols = (vals & 255) + 256 * ((vals >> 8) != H)
            sidx[qq, lcols] = np.arange(len(vals), dtype=np.int16)
            palidx1[qq, : len(vals)] = vals + 1.0
            for s_, v in enumerate(vals):
                rankof[(qq, v)] = s_ + 1
        # rank+1 of each triplet's positive within its row palette
        pbuk = np.zeros((NROW, LCOL), dtype=np.float32)
        pbuk[qs, slot] = np.array(
            [rankof[(qqv, pv)] for qqv, pv in zip(qs, psel[order])], dtype=np.float32
        )
        pbuk = pbuk.astype(ml_bf16)
        ranks1 = np.broadcast_to(
            np.arange(1, s_pad + 1, dtype=np.float32), (NROW, s_pad)
        ).astype(ml_bf16)
        packa = np.concatenate(
            [
                np.ascontiguousarray(samples[perm].T).reshape(2, 128, N).transpose(1, 0, 2).reshape(128, 1024),
                np.ascontiguousarray(samples[rows].T).reshape(2, 128, NROW).transpose(1, 0, 2).reshape(128, 256),
            ],
            axis=1,
        ).astype(np.float32)
        packb = np.concatenate(
            [
                nbuk.view(np.int16) if nbuk.dtype == np.int16 else nbuk,
                sidx,
                pbuk.view(np.int16),
                ranks1.view(np.int16),
            ],
            axis=1,
        )
        in_maps.append({"packa": packa, "packb": packb})
    return in_maps


def kernel(samples, targets, anchor_idx, pos_idx, neg_idx, _want_trace=False):
    samples = np.asarray(samples, dtype=np.float32)
    targets = np.asarray(targets).astype(np.int32)
    a = np.asarray(anchor_idx).astype(np.int64)
    p = np.asarray(pos_idx).astype(np.int64)
    n = np.asarray(neg_idx).astype(np.int64)
    T = a.shape[0]
    assert samples.shape == (N, D)

    ok = (
        np.all((a >= 0) & (a < N) & (p >= 0) & (p < N) & (n >= 0) & (n < N))
        and len(np.unique(a * N + n)) == T
    )
    if not ok:
        raise NotImplementedError("inputs violate mined-triplet structure")

    ap_pairs = np.unique(a * N + p)
    npal = np.bincount(ap_pairs // N, minlength=N)
    s_max = int(npal.max())
    s_pad = max(2, s_max + (s_max & 1))
    if s_pad > 32:
        raise NotImplementedError("palette too large for this kernel")

    nc = _get_program(s_pad)
    in_maps = _shard_inputs(samples, targets, a, p, n, s_pad)
    res = run_bass_kernel_spmd(nc, in_maps, list(range(NCORES)), trace=_want_trace)
    total = sum(float(res.results[c]["out"].astype(np.float64).sum()) for c in range(NCORES))
    loss = np.float32(total / T)
    if _want_trace:
        return loss, res
    return loss



# revision 3
# speedup vs baseline: 1.5993x; 1.5993x over previous
"""Trainium2 Bass kernel for BatchWiseTripletDistanceLoss (v2, banded scatter).

loss = mean_t relu(cos_d(s[a],s[p]) - cos_d(s[a],s[n]) + margin)
     = mean_t relu(sim[a,n] - sim[a,p] + margin)        (the "1-"s cancel)

Each of 8 cores owns 128 anchor rows (a mod 4) and half the negatives
(n >> 8).  On device it computes u = cos(anchor, all 512 samples) + 2 in
f16 via f16 matmuls (dot products + squared norms -> rsqrt outer product),
then one gpsimd local_scatter distributes the u row into
  [palette | banded grid]:
  - palette slot s        <- u at the row's s-th distinct positive column
  - band cells            <- u at each triplet's negative column, with each
    row's triplets packed by positive-slot into width-32 bands (one slot
    per (row, band)).
A second tiny scatter builds cband[row, band] = u_pos of that band's slot.
Then ONE fused DVE op per core computes
    acc1 = sum max(cband - margin, grid),  acc2 = sum cband
and the host combines: sum relu = acc1 - 32*acc2 + 32*margin*n_used_bands
(empty cells/bands cancel exactly since u >= 1 > 0).  Host does layout /
integer metadata only, plus the final tiny partial-sum reduction.
"""
import sys

sys.path.insert(0, "/opt/trn_rl_repo")

from contextlib import ExitStack

import numpy as np

import concourse.bacc as bacc
import concourse.tile as tile
from concourse import mybir
from concourse.bass_utils import run_bass_kernel_spmd

DT = mybir.dt
OP = mybir.AluOpType
ACTF = mybir.ActivationFunctionType

N = 512
D = 256
MARGIN = 0.15
NCORES = 8
NROW = 128
WBAND = 32  # band width (cells per band)


def _build_program(s_pad: int, nband: int, m_span: int):
    """Build + compile the SPMD program (identical for all 8 cores)."""
    nc = bacc.Bacc(
        "TRN2", target_bir_lowering=False, debug=False, num_devices=NCORES
    )
    f32, i16, f16 = DT.float32, DT.int16, DT.float16

    WIDE = s_pad + nband * WBAND
    NI3 = s_pad * m_span
    d_packa = nc.dram_tensor("packa", [128, 1024], f16, kind="ExternalInput").ap()
    d_packb = nc.dram_tensor("packb", [NROW, N + NI3], i16, kind="ExternalInput").ap()
    d_out = nc.dram_tensor("out", [NROW, 2], f32, kind="ExternalOutput").ap()

    with tile.TileContext(nc) as tc, ExitStack() as ctx:
        cpool = ctx.enter_context(tc.tile_pool(name="const", bufs=1))
        wpool = ctx.enter_context(tc.tile_pool(name="work", bufs=1))
        ppool = ctx.enter_context(tc.tile_pool(name="psum", bufs=1, space="PSUM"))

        # ---- input DMAs (two queues) ------------------------------------
        packa = cpool.tile([128, 1024], f16)
        nc.sync.dma_start(packa[:], d_packa)
        packb = cpool.tile([NROW, N + NI3], i16)
        nc.scalar.dma_start(packb[:], d_packb)
        idxs_all = packb[:, 0:N]
        idxs3 = packb[:, N : N + NI3]

        ones_col = cpool.tile([128, 1], f16)
        nc.vector.memset(ones_col[:], 1.0)

        # ---- warmups hidden under the DMA phase -------------------------
        # sqrt ACT table preload
        dumin = cpool.tile([1, 1], f32)
        nc.vector.memset(dumin[:], 4.0)
        dumout = cpool.tile([1, 1], f32)
        nc.scalar.sqrt(dumout[:], dumin[:])
        # gpsimd local_scatter ucode IRAM load (~6us, hidden here)
        dmy_d = cpool.tile([128, 2], f16)
        nc.vector.memset(dmy_d[:], 0.0)
        dmy_i = cpool.tile([128, 2], i16)
        nc.vector.memset(dmy_i[:], -1)
        dmy_o = cpool.tile([128, 2], f16)
        nc.gpsimd.local_scatter(
            dmy_o[:], dmy_d[:], dmy_i[:], channels=128, num_elems=2, num_idxs=2
        )

        st = [packa[:, 0:512], packa[:, 512:1024]]

        # ---- squares + norm sums ----------------------------------------
        sqall = wpool.tile([128, 1024], f16, tag="sqall")
        nc.vector.tensor_tensor(sqall[:], packa[:], packa[:], OP.mult)
        sq = [sqall[:, 0:512], sqall[:, 512:1024]]

        n2p = ppool.tile([1, N], f32, tag="n2p")
        for k in range(2):
            nc.tensor.matmul(n2p[:], ones_col[:], sq[k], start=(k == 0), stop=(k == 1))

        # ---- sim matmul (anchors are the first 128 columns) -------------
        simp = ppool.tile([128, N], f32, tag="simp")
        for k in range(2):
            nc.tensor.matmul(
                simp[:], st[k][:, 0:128], st[k], start=(k == 0), stop=(k == 1)
            )

        # ---- norms -> reciprocal row, f16 -------------------------------
        nrow = wpool.tile([1, N], f32, tag="nrow")
        nc.scalar.sqrt(nrow[:], n2p[:])
        rrow = wpool.tile([1, N], f32, tag="rrow")
        nc.vector.reciprocal_approx_fast(rrow[:], nrow[:])
        rrow16 = wpool.tile([1, N], f16, tag="rrow16")
        nc.scalar.copy(rrow16[:], rrow[:])

        # rbp2[q, j] = (1/|a_q|) * (1/|s_j|)  (outer product on PE)
        rbp2 = ppool.tile([128, N], f32, tag="rbp2")
        nc.tensor.matmul(rbp2[:], rrow16[:, 0:128], rrow16[:], start=True, stop=True)

        # simp -> SBUF (overlaps norm chain)
        simp_sb = wpool.tile([128, N], f32, tag="simp_sb")
        nc.scalar.copy(simp_sb[:], simp[:])

        # u16 = cos + 2 in f16
        t1 = wpool.tile([128, N], f32, tag="t1")
        nc.vector.tensor_tensor(t1[:], simp_sb[:], rbp2[:], OP.mult)
        u16 = wpool.tile([128, N], f16, tag="u16")
        nc.scalar.activation(u16[:], t1[:], ACTF.Copy, bias=2.0)

        # ---- merged scatter: [palette | banded grid] --------------------
        dst = wpool.tile([NROW, WIDE], f16, tag="dst")
        nc.gpsimd.local_scatter(
            dst[:], u16[:], idxs_all, channels=128, num_elems=WIDE, num_idxs=N
        )
        palv = dst[:, 0:s_pad]
        grid = dst[:, s_pad:WIDE]

        # crep = palette values replicated per span slot (exact f16 copy)
        crep = wpool.tile([NROW, NI3], f16, tag="crep")
        nc.vector.tensor_scalar_add(
            crep[:].rearrange("p (s j) -> p s j", s=s_pad),
            palv.unsqueeze(2).to_broadcast((NROW, s_pad, m_span)),
            0.0,
        )
        cband = wpool.tile([NROW, nband], f16, tag="cband")
        nc.gpsimd.local_scatter(
            cband[:], crep[:], idxs3, channels=128, num_elems=nband, num_idxs=NI3
        )

        # ---- fused evaluate + accumulate --------------------------------
        accs = wpool.tile([NROW, 2], f32, tag="accs")
        scr = wpool.tile([NROW, nband * WBAND], f32, tag="scr")
        nc.vector.scalar_tensor_tensor(
            scr[:].rearrange("p (b w) -> p b w", b=nband),
            cband[:].unsqueeze(2).to_broadcast((NROW, nband, WBAND)),
            -MARGIN,
            grid.rearrange("p (b w) -> p b w", b=nband),
            OP.add,
            OP.max,
            accum_out=accs[:, 0:1],
        )
        scr2 = wpool.tile([NROW, nband], f32, tag="scr2")
        nc.scalar.activation(scr2[:], cband[:], ACTF.Copy, accum_out=accs[:, 1:2])

        nc.sync.dma_start(d_out, accs[:])

    nc.compile()
    return nc


_PROGRAM_CACHE = {}


def _get_program(key):
    if key not in _PROGRAM_CACHE:
        _PROGRAM_CACHE[key] = _build_program(*key)
    return _PROGRAM_CACHE[key]


def _core_sel(a, p, n, core):
    R, H = core >> 1, core & 1
    sel = ((a & 3) == R) & ((n >> 8) == H)
    return R, H, a[sel] >> 2, p[sel], n[sel]


def _shard_inputs(samples, a, p, n, s_pad, nband, m_span):
    in_maps = []
    nb_tot = []
    for core in range(NCORES):
        R, H, q, ps, ns = _core_sel(a, p, n, core)
        anchor_rows = np.arange(NROW, dtype=np.int64) * 4 + R
        others = np.setdiff1d(np.arange(N, dtype=np.int64), anchor_rows)
        perm = np.concatenate([anchor_rows, others])
        col_of = np.empty(N, dtype=np.int64)
        col_of[perm] = np.arange(N)

        idxs_all = np.full((NROW, N), -1, dtype=np.int16)
        idxs3 = np.full((NROW, s_pad * m_span), -1, dtype=np.int16)
        nb = 0
        order = np.argsort(q, kind="stable")
        qs, pss, nss = q[order], ps[order], ns[order]
        starts = np.searchsorted(qs, np.arange(NROW + 1))
        for qq in range(NROW):
            lo, hi = starts[qq], starts[qq + 1]
            if lo == hi:
                continue
            pq, nq = pss[lo:hi], nss[lo:hi]
            vals, inv = np.unique(pq, return_inverse=True)
            idxs_all[qq, col_of[vals]] = np.arange(len(vals), dtype=np.int16)
            band_start = 0
            for s in range(len(vals)):
                cols = col_of[nq[inv == s]]
                c = len(cols)
                nbd = -(-c // WBAND)
                idxs_all[qq, cols] = (
                    s_pad + band_start * WBAND + np.arange(c)
                ).astype(np.int16)
                idxs3[qq, s * m_span : s * m_span + nbd] = band_start + np.arange(
                    nbd, dtype=np.int16
                )
                band_start += nbd
            nb += band_start

        packa = (
            np.ascontiguousarray(samples[perm].T)
            .reshape(2, 128, N)
            .transpose(1, 0, 2)
            .reshape(128, 1024)
            .astype(np.float16)
        )
        packb = np.concatenate([idxs_all, idxs3], axis=1)
        in_maps.append({"packa": packa, "packb": packb})
        nb_tot.append(nb)
    return in_maps, nb_tot


def kernel(samples, targets, anchor_idx, pos_idx, neg_idx, _want_trace=False):
    samples = np.asarray(samples, dtype=np.float32)
    a = np.asarray(anchor_idx).astype(np.int64)
    p = np.asarray(pos_idx).astype(np.int64)
    n = np.asarray(neg_idx).astype(np.int64)
    T = a.shape[0]
    assert samples.shape == (N, D)

    ok = (
        np.all((a >= 0) & (a < N) & (p >= 0) & (p < N) & (n >= 0) & (n < N))
        and len(np.unique(a * N + n)) == T
    )
    if not ok:
        raise NotImplementedError("inputs violate mined-triplet structure")

    # layout constants (max over cores)
    s_pad, nband, m_span = 2, 2, 1
    for core in range(NCORES):
        _, _, q, ps, ns = _core_sel(a, p, n, core)
        key = q * N + ps
        uniq, cnt = np.unique(key, return_counts=True)
        rows = uniq // N
        s_pad = max(s_pad, int(np.bincount(rows).max()))
        spans = -(-cnt // WBAND)
        m_span = max(m_span, int(spans.max()))
        nband = max(nband, int(np.bincount(rows, weights=spans).max()))
    s_pad += s_pad & 1
    nband += nband & 1
    if s_pad > 64 or nband * WBAND > 1500:
        raise NotImplementedError("palette/band layout too large")

    key = (s_pad, nband, m_span)
    nc = _get_program(key)
    in_maps, nb_tot = _shard_inputs(samples, a, p, n, s_pad, nband, m_span)
    res = run_bass_kernel_spmd(nc, in_maps, list(range(NCORES)), trace=_want_trace)
    total = 0.0
    for c in range(NCORES):
        o = res.results[c]["out"].astype(np.float64)
        total += float(o[:, 0].sum() - WBAND * o[:, 1].sum())
        total += WBAND * MARGIN * nb_tot[c]
    loss = np.float32(total / T)
    if _want_trace:
        return loss, res
    return loss


# revision 11
# speedup vs baseline: 1.7819x; 1.1142x over previous
"""Trainium2 Bass kernel for BatchWiseTripletDistanceLoss (v2, banded scatter).

loss = mean_t relu(cos_d(s[a],s[p]) - cos_d(s[a],s[n]) + margin)
     = mean_t relu(sim[a,n] - sim[a,p] + margin)        (the "1-"s cancel)

Each of 8 cores owns 128 anchor rows (a mod 4) and half the negatives
(n >> 8).  On device it computes u = cos(anchor, all 512 samples) + 2 in
f16 via f16 matmuls (dot products + squared norms -> rsqrt outer product),
then one gpsimd local_scatter distributes the u row into
  [palette | banded grid]:
  - palette slot s        <- u at the row's s-th distinct positive column
  - band cells            <- u at each triplet's negative column, with each
    row's triplets packed by positive-slot into width-32 bands (one slot
    per (row, band)).
A second tiny scatter builds cband[row, band] = u_pos of that band's slot.
Then ONE fused DVE op per core computes
    acc1 = sum max(cband - margin, grid),  acc2 = sum cband
and the host combines: sum relu = acc1 - 32*acc2 + 32*margin*n_used_bands
(empty cells/bands cancel exactly since u >= 1 > 0).  Host does layout /
integer metadata only, plus the final tiny partial-sum reduction.
"""
import sys

sys.path.insert(0, "/opt/trn_rl_repo")

from contextlib import ExitStack

import numpy as np

import concourse.bacc as bacc
import concourse.tile as tile
from concourse import mybir
from concourse.bass_utils import run_bass_kernel_spmd

DT = mybir.dt
OP = mybir.AluOpType
ACTF = mybir.ActivationFunctionType

N = 512
D = 256
MARGIN = 0.15
NCORES = 8
NROW = 128
WBAND = 32  # band width (cells per band)


def _build_program(s_pad: int, nband: int, m_span: int):
    """Build + compile the SPMD program (identical for all 8 cores)."""
    nc = bacc.Bacc(
        "TRN2", target_bir_lowering=False, debug=False, num_devices=NCORES
    )
    f32, i16, f16 = DT.float32, DT.int16, DT.float16

    WIDE = s_pad + nband * WBAND
    NI3 = s_pad * m_span
    d_packa = nc.dram_tensor("packa", [128, 1024], f16, kind="ExternalInput").ap()
    d_packb = nc.dram_tensor("packb", [NROW, N + NI3], i16, kind="ExternalInput").ap()
    d_out = nc.dram_tensor("out", [NROW, 2], f32, kind="ExternalOutput").ap()

    with tile.TileContext(nc) as tc, ExitStack() as ctx:
        cpool = ctx.enter_context(tc.tile_pool(name="const", bufs=1))
        wpool = ctx.enter_context(tc.tile_pool(name="work", bufs=1))
        ppool = ctx.enter_context(tc.tile_pool(name="psum", bufs=1, space="PSUM"))

        # ---- input DMAs (two queues) ------------------------------------
        packa = cpool.tile([128, 1024], f16)
        nc.sync.dma_start(packa[:], d_packa)
        packb = cpool.tile([NROW, N + NI3], i16)
        nc.scalar.dma_start(packb[:], d_packb)
        idxs_all = packb[:, 0:N]
        idxs3 = packb[:, N : N + NI3]

        ones_col = cpool.tile([128, 1], f16)
        nc.vector.memset(ones_col[:], 1.0)

        # ---- warmups hidden under the DMA phase -------------------------
        # rsqrt ACT table preload (the only ACT table this kernel uses)
        dumin = cpool.tile([1, 1], f32)
        nc.vector.memset(dumin[:], 4.0)
        dumout = cpool.tile([1, 1], f32)
        nc.scalar.activation(dumout[:], dumin[:], ACTF.Abs_reciprocal_sqrt)
        # gpsimd local_scatter ucode IRAM load (~6us, hidden here)
        dmy_d = cpool.tile([128, 2], f16)
        nc.vector.memset(dmy_d[:], 0.0)
        dmy_i = cpool.tile([128, 2], i16)
        nc.vector.memset(dmy_i[:], -1)
        dmy_o = cpool.tile([128, 2], f16)
        nc.gpsimd.local_scatter(
            dmy_o[:], dmy_d[:], dmy_i[:], channels=128, num_elems=2, num_idxs=2
        )

        st = [packa[:, 0:512], packa[:, 512:1024]]

        # ---- squares + norm sums ----------------------------------------
        sqall = wpool.tile([128, 1024], f16, tag="sqall")
        nc.vector.tensor_tensor(sqall[:], packa[:], packa[:], OP.mult)
        sq = [sqall[:, 0:512], sqall[:, 512:1024]]

        n2p = ppool.tile([1, N], f32, tag="n2p")
        for k in range(2):
            nc.tensor.matmul(n2p[:], ones_col[:], sq[k], start=(k == 0), stop=(k == 1))

        # ---- sim matmul (anchors are the first 128 columns) -------------
        simp = ppool.tile([128, N], f32, tag="simp")
        for k in range(2):
            nc.tensor.matmul(
                simp[:], st[k][:, 0:128], st[k], start=(k == 0), stop=(k == 1)
            )

        # ---- norms -> reciprocal row in one table op --------------------
        rrow16 = wpool.tile([1, N], f16, tag="rrow16")
        nc.scalar.activation(rrow16[:], n2p[:], ACTF.Abs_reciprocal_sqrt)

        # rbp2[q, j] = (1/|a_q|) * (1/|s_j|)  (outer product on PE)
        rbp2 = ppool.tile([128, N], f32, tag="rbp2")
        nc.tensor.matmul(rbp2[:], rrow16[:, 0:128], rrow16[:], start=True, stop=True)

        # simp -> SBUF (overlaps norm chain)
        simp_sb = wpool.tile([128, N], f32, tag="simp_sb")
        nc.vector.tensor_scalar_add(simp_sb[:], simp[:], 0.0)

        # u16 = cos + 2 in f16
        t1 = wpool.tile([128, N], f32, tag="t1")
        nc.vector.tensor_tensor(t1[:], simp_sb[:], rbp2[:], OP.mult)
        u16 = wpool.tile([128, N], f16, tag="u16")
        nc.vector.tensor_scalar_add(u16[:], t1[:], 2.0)

        # ---- merged scatter: [palette | banded grid] --------------------
        dst = wpool.tile([NROW, WIDE], f16, tag="dst")
        nc.gpsimd.local_scatter(
            dst[:], u16[:], idxs_all, channels=128, num_elems=WIDE, num_idxs=N
        )
        palv = dst[:, 0:s_pad]
        grid = dst[:, s_pad:WIDE]

        # crep = palette values replicated per span slot (exact f16 copy)
        crep = wpool.tile([NROW, NI3], f16, tag="crep")
        nc.vector.tensor_scalar_add(
            crep[:].rearrange("p (s j) -> p s j", s=s_pad),
            palv.unsqueeze(2).to_broadcast((NROW, s_pad, m_span)),
            0.0,
        )
        cband = wpool.tile([NROW, nband], f16, tag="cband")
        nc.gpsimd.local_scatter(
            cband[:], crep[:], idxs3, channels=128, num_elems=nband, num_idxs=NI3
        )

        # ---- fused evaluate + accumulate --------------------------------
        accs = wpool.tile([NROW, 2], f32, tag="accs")
        scr = wpool.tile([NROW, nband * WBAND], f32, tag="scr")
        nc.vector.scalar_tensor_tensor(
            scr[:].rearrange("p (b w) -> p b w", b=nband),
            cband[:].unsqueeze(2).to_broadcast((NROW, nband, WBAND)),
            -MARGIN,
            grid.rearrange("p (b w) -> p b w", b=nband),
            OP.add,
            OP.max,
            accum_out=accs[:, 0:1],
        )
        scr2 = wpool.tile([NROW, nband], f32, tag="scr2")
        nc.vector.tensor_scalar(
            scr2[:], cband[:], 0.0, 0.0, OP.add, OP.add, accum_out=accs[:, 1:2]
        )

        nc.sync.dma_start(d_out, accs[:])

    nc.compile()
    return nc


_PROGRAM_CACHE = {}


def _get_program(key):
    if key not in _PROGRAM_CACHE:
        _PROGRAM_CACHE[key] = _build_program(*key)
    return _PROGRAM_CACHE[key]


def _core_sel(a, p, n, core):
    R, H = core >> 1, core & 1
    sel = ((a & 3) == R) & ((n >> 8) == H)
    return R, H, a[sel] >> 2, p[sel], n[sel]


def _shard_inputs(samples, a, p, n, s_pad, nband, m_span):
    in_maps = []
    nb_tot = []
    for core in range(NCORES):
        R, H, q, ps, ns = _core_sel(a, p, n, core)
        anchor_rows = np.arange(NROW, dtype=np.int64) * 4 + R
        others = np.setdiff1d(np.arange(N, dtype=np.int64), anchor_rows)
        perm = np.concatenate([anchor_rows, others])
        col_of = np.empty(N, dtype=np.int64)
        col_of[perm] = np.arange(N)

        idxs_all = np.full((NROW, N), -1, dtype=np.int16)
        idxs3 = np.full((NROW, s_pad * m_span), -1, dtype=np.int16)
        nb = 0
        order = np.argsort(q, kind="stable")
        qs, pss, nss = q[order], ps[order], ns[order]
        starts = np.searchsorted(qs, np.arange(NROW + 1))
        for qq in range(NROW):
            lo, hi = starts[qq], starts[qq + 1]
            if lo == hi:
                continue
            pq, nq = pss[lo:hi], nss[lo:hi]
            vals, inv = np.unique(pq, return_inverse=True)
            idxs_all[qq, col_of[vals]] = np.arange(len(vals), dtype=np.int16)
            band_start = 0
            for s in range(len(vals)):
                cols = col_of[nq[inv == s]]
                c = len(cols)
                nbd = -(-c // WBAND)
                idxs_all[qq, cols] = (
                    s_pad + band_start * WBAND + np.arange(c)
                ).astype(np.int16)
                idxs3[qq, s * m_span : s * m_span + nbd] = band_start + np.arange(
                    nbd, dtype=np.int16
                )
                band_start += nbd
            nb += band_start

        packa = (
            np.ascontiguousarray(samples[perm].T)
            .reshape(2, 128, N)
            .transpose(1, 0, 2)
            .reshape(128, 1024)
            .astype(np.float16)
        )
        packb = np.concatenate([idxs_all, idxs3], axis=1)
        in_maps.append({"packa": packa, "packb": packb})
        nb_tot.append(nb)
    return in_maps, nb_tot


def kernel(samples, targets, anchor_idx, pos_idx, neg_idx, _want_trace=False):
    samples = np.asarray(samples, dtype=np.float32)
    a = np.asarray(anchor_idx).astype(np.int64)
    p = np.asarray(pos_idx).astype(np.int64)
    n = np.asarray(neg_idx).astype(np.int64)
    T = a.shape[0]
    assert samples.shape == (N, D)

    ok = (
        np.all((a >= 0) & (a < N) & (p >= 0) & (p < N) & (n >= 0) & (n < N))
        and len(np.unique(a * N + n)) == T
    )
    if not ok:
        raise NotImplementedError("inputs violate mined-triplet structure")

    # layout constants (max over cores)
    s_pad, nband, m_span = 2, 2, 1
    for core in range(NCORES):
        _, _, q, ps, ns = _core_sel(a, p, n, core)
        key = q * N + ps
        uniq, cnt = np.unique(key, return_counts=True)
        rows = uniq // N
        s_pad = max(s_pad, int(np.bincount(rows).max()))
        spans = -(-cnt // WBAND)
        m_span = max(m_span, int(spans.max()))
        nband = max(nband, int(np.bincount(rows, weights=spans).max()))
    s_pad += s_pad & 1
    nband += nband & 1
    if s_pad > 64 or nband * WBAND > 1500:
        raise NotImplementedError("palette/band layout too large")

    key = (s_pad, nband, m_span)
    nc = _get_program(key)
    in_maps, nb_tot = _shard_inputs(samples, a, p, n, s_pad, nband, m_span)
    res = run_bass_kernel_spmd(nc, in_maps, list(range(NCORES)), trace=_want_trace)
    total = 0.0
    for c in range(NCORES):
        o = res.results[c]["out"].astype(np.float64)
        total += float(o[:, 0].sum() - WBAND * o[:, 1].sum())
        total += WBAND * MARGIN * nb_tot[c]
    loss = np.float32(total / T)
    if _want_trace:
        return loss, res
    return loss
